# revision 11
# baseline (speedup 1.0000x reference)
"""Trainium2 Bass kernel for nn_CrossAttention (B=8192, SQ=5, SK=49, H=8, E=256).

Data-parallel over 8 NeuronCores (B_LOC=1024/core). Math restructure:
  Wqk  = Wq @ Wk^T  (per-head 32x32)      -> no separate K projection
  WvWo = blockdiag(Wv) @ Wo (as [(h,p),e']) -> no separate V projection
  scores = (q.Wqk.enc^T + qp.kp^T)/16 ; attn = softmax_k(scores) (no max-sub,
  scores bounded |~2.5|); c = attn @ enc_heads ; out = c @ WvWo.

Implementation notes:
  - inputs cast fp32->bf16 during HBM load (SWDGE cast DMA)
  - enc/kp loaded "padded": one batch elem per 64-partition half so per-batch
    49-row slices are partition-aligned for matmul operands
  - PE transposes -> [E, n] layouts; per-batch attention as K=128/98-chunk
    matmuls with block-diagonal stationary tiles built via SBUF->SBUF DMA
    partition-rearrangement (zeros memset once, persistent double buffers)
"""

import numpy as np
import ml_dtypes
from contextlib import ExitStack

import concourse.bass as bass
import concourse.bacc as bacc
import concourse.tile as tile
from concourse import mybir
from concourse.bass_utils import run_bass_kernel_spmd

BF16 = mybir.dt.bfloat16
FP32 = mybir.dt.float32

H, E, PH, SQ, SK = 8, 256, 32, 5, 49
NQ = H * SQ          # 40
N_CORES = 8


def _cdiv(a, b):
    return (a + b - 1) // b


def build_nc(b_loc=1024, bb=32, debug_taps=False):
    nc = bacc.Bacc("TRN2", target_bir_lowering=False, debug=False)

    d_q = nc.declare_dram_parameter("obj_queries", [b_loc, SQ, E], FP32, isOutput=False)
    d_qp = nc.declare_dram_parameter("query_obj_pos", [b_loc, SQ, E], FP32, isOutput=False)
    d_enc = nc.declare_dram_parameter("encoder_output", [b_loc, SK, E], FP32, isOutput=False)
    d_kp = nc.declare_dram_parameter("key_encoder_pos", [b_loc, SK, E], FP32, isOutput=False)
    d_bdwqk = nc.declare_dram_parameter("BDWQK", [128, 256], BF16, isOutput=False)
    d_wvwo = nc.declare_dram_parameter("WVWO", [128, 512], BF16, isOutput=False)
    d_ident = nc.declare_dram_parameter("IDENT", [128, 128], BF16, isOutput=False)
    d_out = nc.declare_dram_parameter("out", [b_loc, SQ, E], FP32, isOutput=True)

    taps = {}
    if debug_taps:
        npair0 = bb // 2
        for nm, shp in (("t_qcat", [128, 4 * bb * SQ]), ("t_ecat", [128, 4 * npair0 * 113]),
                        ("t_attn", [NQ, bb * SK]), ("t_attnn", [NQ, bb * SK]),
                        ("t_attnT", [113, npair0 * NQ]), ("t_csb", [PH, bb * NQ]),
                        ("t_cT", [128, 2 * bb * SQ]), ("t_sums", [NQ, bb]),
                        ("t_bdq", [128, 4 * bb * NQ]), ("t_abd", [98, 4 * bb * NQ]),
                        ("t_ehs", [98, 4 * bb * PH])):
        # bf16 taps stored as fp32 via cast DMA? simpler: declare bf16-shaped fp32 via gpsimd cast
            taps[nm] = nc.declare_dram_parameter(nm, shp, FP32, isOutput=True)

    assert b_loc % bb == 0 and bb % 2 == 0
    n_bb = b_loc // bb
    npair = bb // 2
    nqr = bb * SQ                        # (b,q) rows per block
    n_qt = _cdiv(nqr, 128)               # q-side n-tiles per block

    with tile.TileContext(nc) as tc, ExitStack() as ctx:
        consts = ctx.enter_context(tc.sbuf_pool(name="consts", bufs=1))
        qn_p = ctx.enter_context(tc.sbuf_pool(name="qn_p", bufs=2))
        en_p = ctx.enter_context(tc.sbuf_pool(name="en_p", bufs=2))
        eT_p = ctx.enter_context(tc.sbuf_pool(name="eT_p", bufs=2))
        qc_p = ctx.enter_context(tc.sbuf_pool(name="qc_p", bufs=2))
        bdq_p = ctx.enter_context(tc.sbuf_pool(name="bdq_p", bufs=1))
        attn_p = ctx.enter_context(tc.sbuf_pool(name="attn_p", bufs=2))
        misc_p = ctx.enter_context(tc.sbuf_pool(name="misc_p", bufs=2))
        ehs_p = ctx.enter_context(tc.sbuf_pool(name="ehs_p", bufs=2))
        out_p = ctx.enter_context(tc.sbuf_pool(name="out_p", bufs=2))

        ps_a = ctx.enter_context(tc.psum_pool(name="ps_a", bufs=2))
        ps_sc = ctx.enter_context(tc.psum_pool(name="ps_sc", bufs=2))
        ps_at = ctx.enter_context(tc.psum_pool(name="ps_at", bufs=2))
        ps_c = ctx.enter_context(tc.psum_pool(name="ps_c", bufs=2))

        bdwqk = consts.tile([128, 256], BF16, name="bdwqk")
        nc.sync.dma_start(bdwqk[:], d_bdwqk[:])
        wvwo = consts.tile([128, 512], BF16, name="wvwo")
        nc.sync.dma_start(wvwo[:], d_wvwo[:])
        ident = consts.tile([128, 128], BF16, name="ident")
        nc.sync.dma_start(ident[:], d_ident[:])

        def cp(i, dst, src):
            # alternate PSUM->SBUF copies between DVE and ACT engines
            if i % 2 == 0:
                nc.vector.tensor_copy(dst, src)
            else:
                nc.scalar.copy(dst, src)

        # persistent block-diagonal stationary buffers (zeros survive reuse)
        bdq2 = [bdq_p.tile([128, 4 * bb * NQ], BF16, name=f"bdq{i}") for i in range(2)]
        abd2 = [bdq_p.tile([98, 4 * bb * NQ], BF16, name=f"abd{i}") for i in range(2)]
        for i in range(2):
            nc.vector.memset(bdq2[i][:], 0.0)
            nc.gpsimd.memset(abd2[i][:], 0.0)

        v_q = d_q[:].rearrange("b s e -> (b s) e")
        v_qp = d_qp[:].rearrange("b s e -> (b s) e")
        v_out = d_out[:].rearrange("b s e -> (b s) e")
        v_enc = d_enc[:].rearrange("(t two) k e -> t two k e", two=2)
        v_kp = d_kp[:].rearrange("(t two) k e -> t two k e", two=2)

        for ib in range(n_bb):
            b0 = ib * bb
            bdq = bdq2[ib % 2]
            abd = abd2[ib % 2]

            # ---- loads (cast fp32 -> bf16 via SWDGE) ----
            qn = qn_p.tile([128, n_qt * 256], BF16, name="qn")
            qpn = qn_p.tile([128, n_qt * 256], BF16, name="qpn")
            for t in range(n_qt):
                r0, r1 = t * 128, min((t + 1) * 128, nqr)
                nc.gpsimd.dma_start(qn[0:r1 - r0, t * 256:(t + 1) * 256],
                                    v_q[b0 * SQ + r0:b0 * SQ + r1, :])
                nc.gpsimd.dma_start(qpn[0:r1 - r0, t * 256:(t + 1) * 256],
                                    v_qp[b0 * SQ + r0:b0 * SQ + r1, :])
            encn = en_p.tile([128, npair * 256], BF16, name="encn")
            kpn = en_p.tile([128, npair * 256], BF16, name="kpn")
            for (vsrc, dst) in ((v_enc, encn), (v_kp, kpn)):
                for par in range(2):
                    nc.gpsimd.dma_start(
                        dst[64 * par:64 * par + SK, :].rearrange(
                            "k (t e) -> k t e", e=256),
                        vsrc[b0 // 2:b0 // 2 + npair, par, :, :].transpose([1, 0, 2]))

            # ---- q-side transposes + Q~ projection -> qcat (bf16 [128, 4*nqr])
            qT = qc_p.tile([128, 2 * nqr], BF16, name="qT")
            qcat = qc_p.tile([128, 4 * nqr], BF16, name="qcat")
            for (src_t, kind) in ((qn, 0), (qpn, 1)):
                for c in range(2):
                    pt = ps_a.tile([128, 1024], BF16, name="pt", tag="psA")
                    for t in range(n_qt):
                        r0, r1 = t * 128, min((t + 1) * 128, nqr)
                        nc.tensor.transpose(
                            pt[:, r0:r1],
                            src_t[0:r1 - r0, t * 256 + c * 128:
                                  t * 256 + (c + 1) * 128],
                            ident[0:r1 - r0, 0:r1 - r0])
                    if kind == 0:
                        dst = qT[:, c * nqr:(c + 1) * nqr]
                    else:
                        dst = qcat[:, (2 + c) * nqr:(3 + c) * nqr]
                    cp(c, dst, pt[:, 0:nqr])
            for c in range(2):
                pq = ps_a.tile([128, 512], FP32, name="pq", tag="psA")
                nc.tensor.matmul(pq[:, 0:nqr], bdwqk[:, c * 128:(c + 1) * 128],
                                 qT[:, c * nqr:(c + 1) * nqr])
                cp(c, qcat[:, c * nqr:(c + 1) * nqr], pq[:, 0:nqr])

            # ---- enc/kp transposes -> ecat [128, (c, t, 113)] (cols 49-63 junk)
            ecat = eT_p.tile([128, 4 * npair * 113], BF16, name="ecat")
            for (src_t, c_pair) in ((encn, (0, 1)), (kpn, (2, 3))):
                for c_loc, c in enumerate(c_pair):
                    pend = 0
                    pe = ps_a.tile([128, 1024], BF16, name="pe", tag="psA")
                    for t in range(npair):
                        nc.tensor.transpose(
                            pe[:, pend * 114:pend * 114 + 113],
                            src_t[0:113, (2 * t + c_loc) * 128:
                                  (2 * t + c_loc) * 128 + 128],
                            ident[0:113, 0:113])
                        pend += 1
                        if pend == 8 or t == npair - 1:
                            cp(t,
                               ecat[:, (c * npair + t + 1 - pend) * 113:
                                    (c * npair + t + 1) * 113].rearrange(
                                   "p (n x) -> p n x", x=113),
                               pe[:, 0:pend * 114].rearrange(
                                   "p (n x) -> p n x", x=114)[:, :, 0:113])
                            if t != npair - 1:
                                pe = ps_a.tile([128, 1024], BF16, name="pe", tag="psA")
                            pend = 0

            # ---- bdQ build (SBUF->SBUF DMA into block-diag positions) ----
            for c in range(4):
                for j in range(4):
                    h = 4 * (c % 2) + j
                    nc.sync.dma_start(
                        bdq[32 * j:32 * j + 32, :].rearrange(
                            "p (c b s) -> p c b s", c=4, s=NQ)[
                            :, c, :, 5 * h:5 * h + 5],
                        qcat[32 * j:32 * j + 32,
                             c * nqr:(c + 1) * nqr].rearrange(
                            "p (b s) -> p b s", s=SQ))

            # ---- scores: per-b 4-chunk accumulation -> [40,49] tiles ----
            n_scb = _cdiv(bb, 16)
            scb = [ps_sc.tile([128, 512], FP32, name="scb", tag="scb")
                   for _ in range(n_scb)]
            for b in range(bb):
                sb, rem = b // 16, b % 16
                base, slot = 64 * (rem // 8), rem % 8
                o_ap = scb[sb][base:base + NQ, slot * SK:(slot + 1) * SK]
                t, par = b // 2, b % 2
                for c in range(4):
                    nc.tensor.matmul(
                        o_ap,
                        bdq[:, (c * bb + b) * NQ:(c * bb + b + 1) * NQ],
                        ecat[:, (c * npair + t) * 113 + 64 * par:
                             (c * npair + t) * 113 + 64 * par + SK],
                        start=(c == 0), stop=(c == 3))

            # ---- softmax (no max-subtraction; 1/16 folded into exp scale) ----
            attn = attn_p.tile([NQ, bb * SK], BF16, name="attn")
            for sb in range(n_scb):
                for pi in range(2):
                    bst = sb * 16 + pi * 8
                    nb = min(8, bb - bst)
                    if nb <= 0:
                        continue
                    nc.scalar.activation(
                        attn[:, bst * SK:(bst + nb) * SK].rearrange(
                            "p (b k) -> p b k", k=SK),
                        scb[sb][64 * pi:64 * pi + NQ, 0:nb * SK].rearrange(
                            "p (b k) -> p b k", k=SK),
                        mybir.ActivationFunctionType.Exp, scale=1.0 / 16.0)
            sums = misc_p.tile([NQ, bb], FP32, name="sums")
            nc.vector.tensor_reduce(
                sums[:], attn[:].rearrange("p (b k) -> p b k", k=SK),
                axis=mybir.AxisListType.X, op=mybir.AluOpType.add)
            recip = misc_p.tile([NQ, bb], FP32, name="recip")
            nc.vector.reciprocal(recip[:], sums[:])
            attn_n = attn_p.tile([NQ, bb * SK], BF16, name="attn_n")
            nc.vector.tensor_mul(
                attn_n[:].rearrange("p (b k) -> p b k", k=SK),
                attn[:].rearrange("p (b k) -> p b k", k=SK),
                recip[:].unsqueeze(2).broadcast_to([NQ, bb, SK]))

            # ---- attn^T via PE transpose ([49,40] tiles, 2 bases x 12 slots) --
            attnT = attn_p.tile([113, npair * NQ], BF16, name="attnT")
            n_atb = _cdiv(npair, 12)
            atb = [ps_at.tile([128, 1024], BF16, name="atb", tag="atb")
                   for _ in range(n_atb)]
            for b in range(bb):
                t, par = b // 2, b % 2
                bank, slot = t // 12, t % 12
                nc.tensor.transpose(
                    atb[bank][64 * par:64 * par + SK, slot * NQ:(slot + 1) * NQ],
                    attn_n[:, b * SK:(b + 1) * SK],
                    ident[0:NQ, 0:NQ])
            for bank in range(n_atb):
                t0, t1 = bank * 12, min(bank * 12 + 12, npair)
                for pi in range(2):
                    cp(pi,
                       attnT[64 * pi:64 * pi + SK, t0 * NQ:t1 * NQ],
                       atb[bank][64 * pi:64 * pi + SK, 0:(t1 - t0) * NQ])

            # ---- attnbd + Ehs builds (SBUF->SBUF DMA) ----
            ehs = ehs_p.tile([98, 4 * bb * PH], BF16, name="ehs")
            for c in range(4):
                for hh in range(2):
                    h = 2 * c + hh
                    for par in range(2):
                        nc.sync.dma_start(
                            abd[49 * hh:49 * hh + SK, :].rearrange(
                                "p (c b s) -> p c b s", c=4, s=NQ)[
                                :, c, par::2, 5 * h:5 * h + 5],
                            attnT[64 * par:64 * par + SK, :].rearrange(
                                "p (t s) -> p t s", s=NQ)[:, :, 5 * h:5 * h + 5])
                        nc.sync.dma_start(
                            ehs[49 * hh:49 * hh + SK, :].rearrange(
                                "p (c b v) -> p c b v", c=4, v=PH)[
                                :, c, par::2, :],
                            encn[64 * par:64 * par + SK, :].rearrange(
                                "p (t e) -> p t e", e=256)[
                                :, :, 32 * h:32 * h + 32])

            # ---- AV: c[b] = attn_b @ enc_heads_b  ([32,40] tiles) ----
            n_cb = _cdiv(bb, 24)
            cbk = [ps_c.tile([128, 512], FP32, name="cbk", tag="cbk")
                   for _ in range(n_cb)]
            for b in range(bb):
                bank, rem = b // 24, b % 24
                base, slot = 64 * (rem // 12), rem % 12
                o_ap = cbk[bank][base:base + PH, slot * NQ:(slot + 1) * NQ]
                for c in range(4):
                    nc.tensor.matmul(
                        o_ap,
                        ehs[:, (c * bb + b) * PH:(c * bb + b + 1) * PH],
                        abd[:, (c * bb + b) * NQ:(c * bb + b + 1) * NQ],
                        start=(c == 0), stop=(c == 3))

            # ---- c -> SBUF; cT build; Wo stage; store ----
            c_sb = misc_p.tile([PH, bb * NQ], BF16, name="c_sb")
            for bank in range(n_cb):
                for bi in range(2):
                    bst = bank * 24 + bi * 12
                    nb = min(12, bb - bst)
                    if nb <= 0:
                        continue
                    cp(bi,
                       c_sb[:, bst * NQ:(bst + nb) * NQ],
                       cbk[bank][64 * bi:64 * bi + PH, 0:nb * NQ])
            cT = misc_p.tile([128, 2 * nqr], BF16, name="cT")
            for h in range(H):
                nc.sync.dma_start(
                    cT[32 * (h % 4):32 * (h % 4) + 32,
                       (h // 4) * nqr:(h // 4 + 1) * nqr].rearrange(
                        "p (b s) -> p b s", s=SQ),
                    c_sb[:, :].rearrange("p (b s) -> p b s", s=NQ)[
                        :, :, 5 * h:5 * h + 5])
            for t in range(n_qt):
                r0, r1 = t * 128, min((t + 1) * 128, nqr)
                po = ps_a.tile([128, 512], FP32, name="po", tag="psA")
                for c in range(2):
                    nc.tensor.matmul(
                        po[0:r1 - r0, 0:256],
                        cT[:, c * nqr + r0:c * nqr + r1],
                        wvwo[:, c * 256:(c + 1) * 256],
                        start=(c == 0), stop=(c == 1))
                osb = out_p.tile([128, 256], FP32, name="osb")
                cp(t, osb[0:r1 - r0, :], po[0:r1 - r0, 0:256])
                nc.sync.dma_start(v_out[b0 * SQ + r0:b0 * SQ + r1, :],
                                  osb[0:r1 - r0, :])

            if debug_taps and ib == 0:
                for nm, src in (("t_qcat", qcat), ("t_ecat", ecat), ("t_attn", attn),
                                ("t_attnn", attn_n), ("t_attnT", attnT),
                                ("t_csb", c_sb), ("t_cT", cT), ("t_sums", sums),
                                ("t_bdq", bdq), ("t_abd", abd), ("t_ehs", ehs)):
                    nc.gpsimd.dma_start(taps[nm][:], src[:])

    nc.compile()
    return nc


# ---------------------------------------------------------------------------
_NC_CACHE = {}


def _get_nc(b_loc, bb):
    key = (b_loc, bb)
    if key not in _NC_CACHE:
        _NC_CACHE[key] = build_nc(b_loc, bb)
    return _NC_CACHE[key]


def _host_consts(Wq, Wk, Wv, Wo):
    Wqk = (Wq @ Wk.T).astype(np.float32)
    bd = np.zeros((128, 256), np.float32)
    for c in range(2):
        for j in range(4):
            bd[32 * j:32 * j + 32, c * 128 + 32 * j:c * 128 + 32 * j + 32] = Wqk
    wvwo = np.einsum("pv,hve->hpe", Wv.astype(np.float32),
                     Wo.astype(np.float32).reshape(H, PH, E)).reshape(E, E)
    wvwo2 = np.concatenate([wvwo[0:128], wvwo[128:256]], axis=1)
    ident = np.eye(128, dtype=np.float32)
    bf = ml_dtypes.bfloat16
    return bd.astype(bf), wvwo2.astype(bf), ident.astype(bf)


def kernel(**inputs):
    b_full = inputs["obj_queries"].shape[0]
    b_loc = b_full // N_CORES
    bb = 32 if b_loc % 32 == 0 else b_loc
    nc = _get_nc(b_loc, bb)

    bd, wvwo, ident = _host_consts(inputs["Wq"], inputs["Wk"],
                                   inputs["Wv"], inputs["Wo"])
    f32 = np.float32
    in_maps = []
    for core in range(N_CORES):
        sl = slice(core * b_loc, (core + 1) * b_loc)
        in_maps.append({
            "obj_queries": np.ascontiguousarray(inputs["obj_queries"][sl], f32),
            "query_obj_pos": np.ascontiguousarray(inputs["query_obj_pos"][sl], f32),
            "encoder_output": np.ascontiguousarray(inputs["encoder_output"][sl], f32),
            "key_encoder_pos": np.ascontiguousarray(inputs["key_encoder_pos"][sl], f32),
            "BDWQK": bd, "WVWO": wvwo, "IDENT": ident,
        })
    res = run_bass_kernel_spmd(nc, in_maps, list(range(N_CORES)))
    out = np.concatenate([res.results[i]["out"] for i in range(N_CORES)], axis=0)
    return np.ascontiguousarray(out, f32)
